# revision 22
# baseline (speedup 1.0000x reference)
"""Trainium2 Bass kernel for nn_BlockBlastValueNet1PmultikernelFlattenned.

Strategy (final, v13)
---------------------
The network is 8 tiny conv branches over an 8x8 board followed by small MLPs.
Because the board has only 64 pixels, every conv branch folds into an affine
map of the 64 board values, and the whole net becomes

    y   = x @ W1 + c1                    # [B, NF]  (NF = 2944, 23 K-tiles)
    ry  = max(y, 0)                      # PSUM->SBUF evacuation
    h   = Lrelu( ry @ W2' + b2f )        # includes the 0.01*y direct path
    g1  = Lrelu( h @ W3 + b3 )
    g2  = Lrelu( g1 @ W4 + b4 )          # fc2 (augmented with a ones column)
    out = g2 @ W5                        # fc3 (bias folded via augmentation)

Machine model (measured from NTFF traces of this backend): a 512-col matmul
issues every ~216ns (1 col/cycle @2.4GHz); matmuls whose stationary tiles
occupy DIFFERENT 64-row groups of the PE array run concurrently; fp8e4
DoubleRow streams at the same column rate but contracts two K-planes per
instruction; a NEFF with no fp32-family instructions gets a 1.2x slower
global clock profile.  Only the Vector and Scalar engines can read PSUM
(~1283 / ~1113 ns per 1024-wide evacuation), which makes the 23-tile y
evacuation the hard floor.  Hence:

* step-1 (K=64) runs in fp16 with two M-tiles row-packed per slot at
  tile_position (0,0)/(64,0) -- both halves stream concurrently.
* step-2 is a single full-row accumulation chain: tiles 0..7 (branch 4,
  which alone contributes ~2.1e-2 of fp8 error vs ~1.4e-2 for everything
  else combined) stay fp16; tiles 8..21 run as seven fp8e4 DoubleRow pairs
  and tile 22 as a single fp8 matmul, halving those tiles' K-passes.
  fp8-destined tiles get a x16 step-1 weight scale so the evacuated values
  land in e4m3's normal range with a plain one-op (add,max) evacuation; the
  step-2 psum carries a x256 scale undone by the h-evacuation's ACT scale.
* the Lrelu between the two big matmuls is decomposed as
  Lrelu(v) = 0.01*v + 0.99*relu(v); the 0.01 direct path rides through
  step-1 for free: tile 22 has identity columns (x>=0 here, so relu passes
  x through) whose W2 rows hold 0.01*(W1@W2).
* the serial tail (h->g1->g2->out) stays f32r -- both for accuracy and to
  keep the fast clock profile -- and is interleaved one stage per slot into
  the NEXT block's stream, popped at slot END so tail ACT ops queue behind
  y evacuations.
* per slot, READY step-2 chain matmuls are emitted BEFORE the (possibly
  PSUM-recycle-blocked) step-1 matmuls so the in-order PE queue always has
  work while evacuations catch up.

Data-parallel over 8 NeuronCores (batch 65536 -> 8192/core), 16 blocks of
512 samples: 512-wide psum tiles are ONE bank each, so ps1p bufs=6 gives a
6-deep producer/evacuation pipeline (vs 3 at 1024-wide) and ps2 double-
buffers in the freed bank, eliminating the h-evacuation WAR stall at every
block boundary.  Features on SBUF partitions, samples streaming; matmuls
and evacuations 512 wide, evacuations split Vector/Scalar.
Measured: 187.3us (baseline 259.9us), rel err 1.32e-2 (gate 2e-2).
"""

import numpy as np

# ---------------------------------------------------------------- constants
SPECS = [(1, 1, 1, 0, 0), (2, 2, 6, 1, 1), (3, 3, 8, 1, 1), (4, 4, 8, 2, 2),
         (5, 5, 16, 2, 2), (8, 8, 32, 0, 0), (1, 8, 4, 0, 0), (8, 1, 4, 0, 0)]
BOARD = 8
B_TOTAL = 65536
N_CORES = 8
BC = B_TOTAL // N_CORES          # 8192 samples per core
BLK = 512                        # samples per block (evac width)
CHUNK = 512                      # matmul moving width (1 psum bank fp32)
N_BLK = BC // BLK                # 16

GROUPS = [[4, 5], [3, 6], [2, 7], [1, 0]]   # historical k-order (kept)
_BR_N = []
for kh, kw, fs, ph, pw in SPECS:
    _BR_N.append((BOARD + 2 * ph - kh + 1) * (BOARD + 2 * pw - kw + 1) * fs)
_NF_TRUE = sum(_BR_N)            # 2830
KT = -(-_NF_TRUE // 128)         # 23 K-tiles
NF = KT * 128                    # 2944
N_S1 = (KT + 1) // 2             # 12 row-packed step-1 slots
XC0 = _NF_TRUE + 2               # x-carry column start (tile 22, col 16)
assert XC0 + 64 <= NF

LRELU_NEG = 0.01
NT16 = 8                         # tiles 0..7 (branch 4) stay fp16 in step-2
NPAIR = 7                        # tiles 8..21 as fp8 DoubleRow pairs
# tile 22 runs as a single fp8 matmul
SY = 16.0                        # fp8 tiles: psum/evac scale
PS2 = 256.0                      # step-2 psum scale

# y-evac engine assignment per tile index: V=vector, A=scalar
EVAC_PAT = "VAVAVAVAVAVAVAVAVAVAVVA"      # 23 chars: 12 V, 11 A
assert len(EVAC_PAT) == KT
CHAIN_LAG = 3                    # step-2 chain lags step-1 slots by this


# ---------------------------------------------------------------- host fold
def _fold_params(p):
    """Fold conv branches + MLPs into the dense pipeline weights."""
    n_of = _BR_N
    W1_of, c1_of = {}, {}
    for i, (kh, kw, fs, ph, pw) in enumerate(SPECS):
        Ho = BOARD + 2 * ph - kh + 1
        Wo = BOARD + 2 * pw - kw + 1
        cw = np.asarray(p[f"b{i}_cw"], np.float64)
        cb = np.asarray(p[f"b{i}_cb"], np.float64)
        W1 = np.zeros((64, n_of[i]))
        c1 = np.zeros((n_of[i],))
        for f in range(fs):
            for oh in range(Ho):
                for ow in range(Wo):
                    oi = (f * Ho + oh) * Wo + ow
                    c1[oi] += cb[f]
                    for u in range(kh):
                        for v in range(kw):
                            r, c = oh + u - ph, ow + v - pw
                            w = cw[f, 0, u, v]
                            if 0 <= r < 8 and 0 <= c < 8:
                                W1[r * 8 + c, oi] += w
                            else:
                                c1[oi] += w        # pad value is 1.0
        W1_of[i] = W1
        c1_of[i] = c1

    K_start = {}
    off = 0
    for g in GROUPS:
        for b in g:
            K_start[b] = off
            off += n_of[b]
    assert off == _NF_TRUE
    border = [b for g in GROUPS for b in g]       # h block order
    hpos = {b: j * 16 for j, b in enumerate(border)}

    W1p = np.zeros((64, NF))
    c1p = np.zeros((NF,))
    W2p = np.zeros((NF, 128))
    b2p = np.zeros((128,))
    for b in range(8):
        s, n, hp = K_start[b], n_of[b], hpos[b]
        W1p[:, s:s + n] = W1_of[b]
        c1p[s:s + n] = c1_of[b]
        W2p[s:s + n, hp:hp + 16] = np.asarray(p[f"b{b}_w1"], np.float64).T
        b2p[hp:hp + 16] = np.asarray(p[f"b{b}_b1"], np.float64)

    Wb = np.zeros((128, 64))
    bb = np.zeros((64,))
    for b in range(8):
        hp = hpos[b]
        Wb[hp:hp + 16, 8 * b:8 * b + 8] = np.asarray(p[f"b{b}_w2"], np.float64).T
        bb[8 * b:8 * b + 8] = np.asarray(p[f"b{b}_b2"], np.float64)
    fc_w1 = np.asarray(p["fc_w1"], np.float64)
    fc_b1 = np.asarray(p["fc_b1"], np.float64)
    W3 = Wb @ fc_w1.T
    b3 = bb @ fc_w1.T + fc_b1
    fc_w2 = np.asarray(p["fc_w2"], np.float64)
    fc_b2 = np.asarray(p["fc_b2"], np.float64)
    fc_w3 = np.asarray(p["fc_w3"], np.float64)
    fc_b3 = np.asarray(p["fc_b3"], np.float64)
    W4 = np.zeros((64, 17)); W4[:, :16] = fc_w2.T
    b4 = np.zeros((17,)); b4[:16] = fc_b2; b4[16] = 1.0
    W5 = np.zeros((17, 1)); W5[:16, 0] = fc_w3[0]; W5[16, 0] = fc_b3[0]

    # Lrelu(v) = 0.01*v + 0.99*relu(v): relu path in W2s, direct path in W12
    W2s = (1.0 - LRELU_NEG) * W2p
    W12 = LRELU_NEG * (W1p @ W2p)                  # [64, 128]
    b2f = LRELU_NEG * (c1p @ W2p) + b2p

    # step-1 augmented W1: x-carry identity columns in tile 22
    W1a = W1p.copy()
    W1a[:, XC0:XC0 + 64] = np.eye(64)
    c1a = c1p.copy()
    c1a[XC0:XC0 + 64] = 0.0
    W2f = W2s.copy()
    W2f[XC0:XC0 + 64, :] = W12

    f32 = np.float32
    f16 = np.float16
    import ml_dtypes
    F8NP = ml_dtypes.float8_e4m3
    # fp8-designated step-2 tiles get a x16 step-1 scale so the evacuated
    # fp8 values land in e4m3's normal range without an extra scale op
    W1s = W1a.copy()
    c1s = c1a.copy()
    W1s[:, 128 * NT16:] *= SY
    c1s[128 * NT16:] *= SY
    dev = {}
    # step-1 weights row-packed: slot s holds M-tiles 2s | 2s+1
    w1 = np.zeros((128, N_S1, 128), f16)
    for s in range(N_S1):
        w1[0:64, s, :] = W1s[:, 128 * (2 * s):128 * (2 * s + 1)]
        if 2 * s + 1 < KT:
            w1[64:128, s, :] = W1s[:, 128 * (2 * s + 1):128 * (2 * s + 2)]
    dev["w1"] = w1
    c1t = np.zeros((128, KT), f32)
    for t in range(KT):
        c1t[:, t] = c1s[128 * t:128 * (t + 1)]
    dev["c1t"] = c1t
    # step-2: psum scale PS2; fp16 tiles carry PS2*W2s, fp8 tiles carry
    # fp8(PS2/SY * W2 rows) against y8 = fp8(SY*ry)
    w2 = np.zeros((128, NT16, 128), f16)
    for t in range(NT16):
        w2[:, t, :] = PS2 * W2f[128 * t:128 * (t + 1), :]
    dev["w2"] = w2
    w2dr = np.zeros((128, NPAIR, 2, 128), F8NP)
    for j in range(NPAIR):
        for pl in range(2):
            t = NT16 + 2 * j + pl
            w2dr[:, j, pl, :] = ((PS2 / SY)
                                 * W2f[128 * t:128 * (t + 1), :]).astype(F8NP)
    dev["w2dr"] = w2dr
    dev["w2s"] = ((PS2 / SY) * W2f[128 * (KT - 1):, :]).astype(F8NP)
    dev["b2f"] = b2f.reshape(128, 1).astype(f32)
    dev["w3"] = W3.astype(f32)
    dev["b3"] = b3.reshape(64, 1).astype(f32)
    dev["w4"] = W4.astype(f32)
    dev["b4"] = b4.reshape(17, 1).astype(f32)
    dev["w5"] = W5.astype(f32)
    return dev


# ---------------------------------------------------------------- device IR
def _build_nc(n_blk=N_BLK):
    import concourse.mybir as mybir
    import concourse.tile as tile
    from concourse import bacc
    from contextlib import ExitStack

    dt = mybir.dt
    AF = mybir.ActivationFunctionType
    ALU = mybir.AluOpType
    f32 = dt.float32
    f32r = dt.float32r
    f16 = dt.float16
    f8 = dt.float8e4
    PM = mybir.MatmulPerfMode
    bc = n_blk * BLK

    nc = bacc.Bacc("TRN2", target_bir_lowering=False, debug=False,
                   num_devices=N_CORES)

    xx_d = nc.dram_tensor("xx", [128, n_blk, BLK], f16, kind="ExternalInput")
    w1_d = nc.dram_tensor("w1", [128, N_S1, 128], f16, kind="ExternalInput")
    c1t_d = nc.dram_tensor("c1t", [128, KT], f32, kind="ExternalInput")
    w2_d = nc.dram_tensor("w2", [128, NT16, 128], f16, kind="ExternalInput")
    w2dr_d = nc.dram_tensor("w2dr", [128, NPAIR, 2, 128], f8,
                            kind="ExternalInput")
    w2s_d = nc.dram_tensor("w2s", [128, 128], f8, kind="ExternalInput")
    b2f_d = nc.dram_tensor("b2f", [128, 1], f32, kind="ExternalInput")
    w3_d = nc.dram_tensor("w3", [128, 64], f32, kind="ExternalInput")
    b3_d = nc.dram_tensor("b3", [64, 1], f32, kind="ExternalInput")
    w4_d = nc.dram_tensor("w4", [64, 17], f32, kind="ExternalInput")
    b4_d = nc.dram_tensor("b4", [17, 1], f32, kind="ExternalInput")
    w5_d = nc.dram_tensor("w5", [17, 1], f32, kind="ExternalInput")
    o_d = nc.dram_tensor("o", [1, bc], f32, kind="ExternalOutput")

    with tile.TileContext(nc) as tc, ExitStack() as ctx:
        wpool = ctx.enter_context(tc.tile_pool(name="wpool", bufs=1))
        xpool = ctx.enter_context(tc.tile_pool(name="xpool", bufs=3))
        ypool = ctx.enter_context(tc.tile_pool(name="ypool", bufs=KT + 3))
        spool = ctx.enter_context(tc.tile_pool(name="spool", bufs=3))
        ps1p = ctx.enter_context(tc.tile_pool(name="ps1p", bufs=6, space="PSUM"))
        ps2p = ctx.enter_context(tc.tile_pool(name="ps2p", bufs=2, space="PSUM"))

        # block-0 input first so compute can start while the rest streams in
        xx_first = xpool.tile([128, BLK], f16, tag="xx", name="xx_first")
        nc.sync.dma_start(xx_first[:], xx_d[:, 0, :])
        w1_t = wpool.tile([128, N_S1, 128], f16)
        nc.gpsimd.dma_start(w1_t[:], w1_d[:])
        c1t_t = wpool.tile([128, KT], f32)
        nc.gpsimd.dma_start(c1t_t[:], c1t_d[:])
        w2_t = wpool.tile([128, NT16, 128], f16)
        nc.gpsimd.dma_start(w2_t[:], w2_d[:])
        w2dr_t = wpool.tile([128, NPAIR, 2, 128], f8)
        nc.gpsimd.dma_start(w2dr_t[:], w2dr_d[:])
        w2s_t = wpool.tile([128, 128], f8)
        nc.gpsimd.dma_start(w2s_t[:], w2s_d[:])
        b2f_t = wpool.tile([128, 1], f32)
        nc.gpsimd.dma_start(b2f_t[:], b2f_d[:])
        w3_t = wpool.tile([128, 64], f32r)
        nc.gpsimd.dma_start(w3_t[:], w3_d[:].bitcast(f32r))
        b3_t = wpool.tile([64, 1], f32)
        nc.gpsimd.dma_start(b3_t[:], b3_d[:])
        w4_t = wpool.tile([64, 17], f32r)
        nc.gpsimd.dma_start(w4_t[:], w4_d[:].bitcast(f32r))
        b4_t = wpool.tile([17, 1], f32)
        nc.gpsimd.dma_start(b4_t[:], b4_d[:])
        w5_t = wpool.tile([17, 1], f32r)
        nc.gpsimd.dma_start(w5_t[:], w5_d[:].bitcast(f32r))

        def make_tail_stages(b, ps2):
            """Per-block serial tail (h -> g1 -> g2 -> out), interleaved into
            the NEXT block's slot stream."""
            st = {}

            def s0():
                st["h"] = spool.tile([128, BLK], f32r, tag="h", name=f"h_{b}")
                nc.scalar.activation(st["h"][:], ps2[:], AF.Lrelu,
                                     bias=b2f_t[:, 0:1], scale=1.0 / PS2,
                                     alpha=LRELU_NEG)

            def s1():
                st["g1ps"] = ps1p.tile([64, BLK], f32, tag="ps1",
                                       name=f"g1ps_{b}")
                for hh in range(BLK // CHUNK):
                    sl = slice(hh * CHUNK, (hh + 1) * CHUNK)
                    nc.tensor.matmul(st["g1ps"][:, sl], w3_t[:],
                                     st["h"][:, sl], start=True, stop=True)

            def s2():
                st["g1"] = spool.tile([64, BLK], f32r, tag="g1", name=f"g1_{b}")
                nc.scalar.activation(st["g1"][:], st["g1ps"][:], AF.Lrelu,
                                     bias=b3_t[:, 0:1], alpha=LRELU_NEG)

            def s3():
                st["g2ps"] = ps1p.tile([17, BLK], f32, tag="ps1",
                                       name=f"g2ps_{b}")
                for hh in range(BLK // CHUNK):
                    sl = slice(hh * CHUNK, (hh + 1) * CHUNK)
                    nc.tensor.matmul(st["g2ps"][:, sl], w4_t[:],
                                     st["g1"][:, sl], start=True, stop=True)

            def s4():
                st["g2"] = spool.tile([17, BLK], f32r, tag="g2", name=f"g2_{b}")
                nc.scalar.activation(st["g2"][:], st["g2ps"][:], AF.Lrelu,
                                     bias=b4_t[:, 0:1], alpha=LRELU_NEG)

            def s5():
                st["ops"] = ps1p.tile([1, BLK], f32, tag="ps1", name=f"ops_{b}")
                for hh in range(BLK // CHUNK):
                    sl = slice(hh * CHUNK, (hh + 1) * CHUNK)
                    nc.tensor.matmul(st["ops"][:, sl], w5_t[:],
                                     st["g2"][:, sl], start=True, stop=True)

            def s6():
                o_t = spool.tile([1, BLK], f32, tag="o", name=f"o_{b}")
                nc.vector.tensor_copy(o_t[:], st["ops"][:])
                nc.sync.dma_start(o_d[:, b * BLK:(b + 1) * BLK], o_t[:])

            return [s0, s1, s2, s3, s4, s5, s6]

        tail_stages = []

        for b in range(n_blk):
            if b == 0:
                xx_t = xx_first
            else:
                xx_t = xpool.tile([128, BLK], f16, tag="xx", name=f"xx_{b}")
                nc.sync.dma_start(xx_t[:], xx_d[:, b, :])

            ps2 = ps2p.tile([128, BLK], f32, tag="ps2", name=f"ps2_{b}")

            ytiles = [None] * KT
            ypairs = [ypool.tile([128, 2, BLK], f8, tag="yp", name=f"yp_{b}_{j}")
                      for j in range(NPAIR)]
            y22 = ypool.tile([128, BLK], f8, tag="yp", name=f"y22_{b}")

            def _chain(t):
                if t < NT16:
                    for hh in range(BLK // CHUNK):
                        sl = slice(hh * CHUNK, (hh + 1) * CHUNK)
                        nc.tensor.matmul(
                            ps2[:, sl], w2_t[:, t, :], ytiles[t][:, sl],
                            start=(t == 0), stop=False,
                            skip_group_check=True)
                elif t == KT - 1:
                    for hh in range(BLK // CHUNK):
                        sl = slice(hh * CHUNK, (hh + 1) * CHUNK)
                        nc.tensor.matmul(
                            ps2[:, sl], w2s_t[:], y22[:, sl],
                            start=False, stop=True,
                            skip_group_check=True)
                elif (t - NT16) % 2 == 1:
                    j = (t - NT16) // 2
                    for hh in range(BLK // CHUNK):
                        sl = slice(hh * CHUNK, (hh + 1) * CHUNK)
                        nc.tensor.matmul(
                            ps2[:, sl], w2dr_t[:, j, :, :],
                            ypairs[j][:, :, sl],
                            start=False, stop=False,
                            perf_mode=PM.DoubleRow,
                            skip_group_check=True)

            pending = []

            for s in range(N_S1):
                if len(pending) > CHAIN_LAG:
                    for t in pending.pop(0):
                        _chain(t)
                tA, tB = 2 * s, 2 * s + 1
                psA = ps1p.tile([128, BLK], f32, tag="ps1", name=f"psA_{b}_{s}")
                if tB < KT:
                    psB = ps1p.tile([128, BLK], f32, tag="ps1",
                                    name=f"psB_{b}_{s}")
                # interleave chunks so the two row-group halves overlap
                for hh in range(BLK // CHUNK):
                    sl = slice(hh * CHUNK, (hh + 1) * CHUNK)
                    nc.tensor.matmul(
                        psA[:, sl], w1_t[0:64, s, :], xx_t[0:64, sl],
                        start=True, stop=True, tile_position=(0, 0))
                    if tB < KT:
                        nc.tensor.matmul(
                            psB[:, sl], w1_t[64:128, s, :], xx_t[64:128, sl],
                            start=True, stop=True, tile_position=(64, 0))
                done = []
                for t, ps in ((tA, psA),) + (((tB, psB),) if tB < KT else ()):
                    if t < NT16:
                        y_t = ypool.tile([128, BLK], f16, tag="y",
                                         name=f"y_{b}_{t}")
                        dst = y_t[:]
                        ytiles[t] = y_t
                    elif t == KT - 1:
                        dst = y22[:]
                    else:
                        j, pl = (t - NT16) // 2, (t - NT16) % 2
                        dst = ypairs[j][:, pl, :]
                    if EVAC_PAT[t] == "V":
                        nc.vector.tensor_scalar(
                            dst, ps[:], c1t_t[:, t:t + 1], 0.0,
                            ALU.add, ALU.max)
                    else:
                        nc.scalar.activation(
                            dst, ps[:], AF.Relu, bias=c1t_t[:, t:t + 1])
                    done.append(t)
                pending.append(done)
                if tail_stages:
                    tail_stages.pop(0)()
            for dd in pending:
                for t in dd:
                    _chain(t)

            for st in tail_stages:
                st()
            tail_stages = make_tail_stages(b, ps2)

        for st in tail_stages:
            st()

    nc.compile()
    return nc


# ---------------------------------------------------------------- execution
_NC_CACHE = {}
LAST_RESULT = None


def _prep_inputs(inputs):
    board = np.asarray(inputs["board"], np.float32).reshape(B_TOTAL, 64)
    x16 = board.astype(np.float16)
    dev = _fold_params(inputs)
    in_maps = []
    for c in range(N_CORES):
        xc = np.ascontiguousarray(x16[c * BC:(c + 1) * BC].T)    # [64, BC]
        xx = np.zeros((128, N_BLK, BLK), np.float16)
        xT = xc.reshape(64, N_BLK, BLK)
        xx[0:64] = xT
        xx[64:128] = xT
        m = dict(dev)
        m["xx"] = xx
        in_maps.append(m)
    return in_maps


def kernel(**inputs):
    global LAST_RESULT
    from concourse.bass_utils import run_bass_kernel_spmd

    if "nc" not in _NC_CACHE:
        _NC_CACHE["nc"] = _build_nc()
    nc = _NC_CACHE["nc"]

    in_maps = _prep_inputs(inputs)
    res = run_bass_kernel_spmd(nc, in_maps, core_ids=list(range(N_CORES)))
    LAST_RESULT = res
    out = np.concatenate([r["o"].reshape(-1) for r in res.results])
    return out.reshape(B_TOTAL, 1).astype(np.float32)
